# revision 13
# baseline (speedup 1.0000x reference)
"""Trainium2 Bass kernel for nn_CausalFullAttention_37821482009327.

Causal full attention (no softmax) with data-dependent complex relative
position decay, silu gating, and output projection.

Sharding: tensor-parallel over the 16 heads -> 2 heads per NeuronCore x 8.
Each core computes its heads' attention and a partial out-projection
(contraction over its 128-wide dim_inner slice); the host sums the 8
partials (the "all-reduce" happens at gather time).

v2 vs baseline:
  - x is transposed on the HOST: device receives XT [D, N]; all 128
    per-core PE transposes of x (and their PSUM evacuations) vanish.
  - attention in bf16 (q,k,ss,v): validated on CPU, ~2.4e-3 max rel err.
  - sim row-packs the 2 heads at PE rows 0-63/64-127 and av col-packs
    them at cols 0-63/64-127 (tile_position) -> both heads' matmuls run
    concurrently in the array.
  - causal trimming: matmuls only stream i >= diagonal; the triangular
    mask is applied in-place on the diagonal 128x128 block via gpsimd
    affine_select (no mask multiplies on DVE).
  - single fp32 x^T tile, bitcast to f32r for q/k/v/g (no CAST copies).

Shapes (hardcoded): B=1, N=2048, D=1024, H=16, Dh=64, Dc=32.
"""
import sys

sys.path.insert(0, "/opt/trn_rl_repo")

import numpy as np

import concourse.bass as bass
import concourse.tile as tile
from concourse import bacc, mybir
from concourse.bass_utils import run_bass_kernel_spmd
from concourse.masks import make_identity

F32 = mybir.dt.float32
F32R = mybir.dt.float32r  # TF32-class matmul fast path (1 cyc/row vs 4)
BF16 = mybir.dt.bfloat16

N = 2048
D = 1024
H_LOC = 2          # heads per core
DH = 64
DC = 32
NCORES = 8
EPS = 1e-10

DCH = D // 128     # 8 d-chunks of 128
NC4 = N // 512     # 4 n-chunks of 512


def _emit(nc):
    """Per-core program (SPMD: same program, per-core weight data).

    Pipeline over 512-row chunks c4=0..3: a-projection (fp32) ->
    complex cumprod scan -> acr -> v/g/q/k projections (f32r) ->
    causal attention (bf16, head-packed) -> partial out-projection.
    """
    XT = nc.dram_tensor("XT", [D, N], F32R, kind="ExternalInput")
    WQ = nc.dram_tensor("WQ", [D, 128], F32R, kind="ExternalInput")
    WK = nc.dram_tensor("WK", [D, 128], F32R, kind="ExternalInput")
    WA = nc.dram_tensor("WA", [D, 128], F32, kind="ExternalInput")
    WV = nc.dram_tensor("WV", [D, 128], F32R, kind="ExternalInput")
    WG = nc.dram_tensor("WG", [D, 128], F32R, kind="ExternalInput")
    WO = nc.dram_tensor("WO", [128, D], F32R, kind="ExternalInput")
    BT = nc.dram_tensor("BT", [128, 8], F32, kind="ExternalInput")
    OUT = nc.dram_tensor("OUT", [D, N], F32, kind="ExternalOutput")

    with (
        tile.TileContext(nc) as tc,
        tc.tile_pool(name="pers", bufs=1) as pers,
        tc.tile_pool(name="ps", bufs=2, space="PSUM") as ps,
        tc.tile_pool(name="xtp", bufs=4) as xtp,
        tc.tile_pool(name="wap", bufs=1) as wap,
        tc.tile_pool(name="scan", bufs=2) as scan,
        tc.tile_pool(name="aep", bufs=2) as aep,
        tc.tile_pool(name="sse", bufs=4) as sse,
        tc.tile_pool(name="chk", bufs=2) as chk,
        tc.tile_pool(name="gte", bufs=2) as gte,
        tc.tile_pool(name="ote", bufs=3) as ote,
    ):
        # ---- persistent SBUF tensors ----
        ident = pers.tile([128, 128], F32, tag="ident")
        make_identity(nc, ident[:])

        kt = pers.tile([128, N], F32R, tag="kt")   # [h*64+d, n] k^T scaled
        vb = pers.tile([128, N], F32R, tag="vb")   # v natural [j_lo][jc*128 + h*64 + d]

        wq_t = pers.tile([128, D], F32R, tag="wq_t")
        wk_t = pers.tile([128, D], F32R, tag="wk_t")
        wg_t = pers.tile([128, D], F32R, tag="wg_t")
        wv_t = pers.tile([128, D], F32R, tag="wv_t")
        wo_t = pers.tile([128, D], F32R, tag="wo_t")
        btile = pers.tile([128, 8], F32, tag="btile")
        for wt, WT in ((wq_t, WQ), (wk_t, WK), (wg_t, WG), (wv_t, WV)):
            nc.sync.dma_start(
                wt[:].rearrange("p (dc c) -> p dc c", dc=DCH),
                WT[:].rearrange("(dc p) c -> p dc c", p=128))
        nc.sync.dma_start(wo_t[:], WO[:])
        nc.sync.dma_start(btile[:], BT[:])
        wq_sb = [wq_t[:, dc * 128:(dc + 1) * 128] for dc in range(DCH)]
        wk_sb = [wk_t[:, dc * 128:(dc + 1) * 128] for dc in range(DCH)]
        wg_sb = [wg_t[:, dc * 128:(dc + 1) * 128] for dc in range(DCH)]
        wv_sb = [wv_t[:, dc * 128:(dc + 1) * 128] for dc in range(DCH)]
        wo_sb = [wo_t[:, ji * 128:(ji + 1) * 128] for ji in range(DCH)]
        wa_sb = [wap.tile([128, 128], F32, tag=f"wa{dc}", name=f"wa{dc}")
                 for dc in range(DCH)]
        for dc in range(DCH):
            nc.sync.dma_start(wa_sb[dc][:], WA[dc * 128:(dc + 1) * 128, :])

        # ---- PE warmup: keep the HAM busy while initial DMAs stream ----
        warm = ps.tile([128, 512], F32, tag="ptr", name="warm", bufs=1)
        for _ in range(20):
            nc.tensor.transpose(warm[:, 0:128], ident[:], ident[:])

        state = {}

        def stage1(c4):
            ns = slice(c4 * 512, (c4 + 1) * 512)

            # x^T chunk arrives pre-transposed from HBM: [128, (dc, 512)]
            xt = xtp.tile([128, DCH * 512], F32R, tag="xt", name="xt")
            xt3 = xt[:].rearrange("p (dc n) -> p dc n", dc=DCH)
            src = XT[:].rearrange("(dc p) n -> p dc n", p=128)[:, :, ns]
            nc.sync.dma_start(xt3[:, 0:4], src[:, 0:4])
            nc.sync.dma_start(xt3[:, 4:8], src[:, 4:8])
            xsr = [xt3[:, dc, :] for dc in range(DCH)]
            xs = [x.bitcast(F32) for x in xsr]  # exact fp32 bits for a-proj

            # ---- 1. fp32 a-projection (a^T = WA^T @ x^T chunk) ----
            pa4 = ps.tile([128, 512], F32, tag="acc", name="pa4")
            for dc in range(DCH):
                nc.tensor.matmul(pa4[:], wa_sb[dc][:], xs[dc],
                                 start=(dc == 0), stop=(dc == DCH - 1))
            at_sb = aep.tile([128, 512], F32, tag="at_sb", name="at_sb")
            nc.scalar.copy(at_sb[:], pa4[:])
            # transpose aT back to natural [n, (c h d)]
            pan = ps.tile([128, 512], F32, tag="ptr", name="pan", bufs=1)
            for s in range(4):
                nc.tensor.transpose(
                    pan[:, s * 128:(s + 1) * 128],
                    at_sb[:, s * 128:(s + 1) * 128], ident[:])
            # scan chunk buffers [128, 256] = [s 4][h 2][d 32]
            reA = scan.tile([128, 256], F32, tag="reA", name="reA", bufs=4)
            imA = scan.tile([128, 256], F32, tag="imA", name="imA", bufs=4)
            reB = scan.tile([128, 256], F32, tag="reB", name="reB")
            imB = scan.tile([128, 256], F32, tag="imB", name="imB")
            t1 = scan.tile([128, 256], F32, tag="t1", name="t1", bufs=1)
            t2 = scan.tile([128, 256], F32, tag="t2", name="t2", bufs=1)
            t3 = scan.tile([128, 256], F32, tag="t3", name="t3", bufs=1)
            t4 = scan.tile([128, 256], F32, tag="t4", name="t4", bufs=1)
            # W_a cols permuted on host to [c(2), h(2), d(32)] per n-block
            src = pan[:].rearrange("p (s c hd) -> p s c hd", s=4, c=2)
            nc.vector.tensor_copy(
                reA[:].rearrange("p (s hd) -> p s hd", s=4), src[:, :, 0])
            nc.vector.tensor_copy(
                imA[:].rearrange("p (s hd) -> p s hd", s=4), src[:, :, 1])

            state[c4] = (xt, xsr, reA, imA, reB, imB, t1, t2, t3, t4)

        def stage2(c4):
            ns = slice(c4 * 512, (c4 + 1) * 512)
            xt, xsr, reA, imA, reB, imB, t1, t2, t3, t4 = state.pop(c4)
            # ---- 2. pointwise: ac = a * sigmoid(|a|)/|a| ----
            nc.gpsimd.tensor_mul(t1[:], reA[:], reA[:])
            nc.vector.tensor_mul(t2[:], imA[:], imA[:])
            nc.vector.tensor_add(t1[:], t1[:], t2[:])          # |a|^2
            nc.scalar.activation(t2[:], t1[:], mybir.ActivationFunctionType.Sqrt)
            nc.vector.reciprocal_approx_fast(t1[:], t2[:])     # 1/|a|
            nc.scalar.activation(t2[:], t2[:],
                                 mybir.ActivationFunctionType.Sigmoid)
            nc.vector.tensor_mul(t1[:], t1[:], t2[:])          # sig(|a|)/|a|
            nc.gpsimd.tensor_mul(reA[:], reA[:], t1[:])
            nc.vector.tensor_mul(imA[:], imA[:], t1[:])

            # ---- 3. doubling scan (complex cumprod over d) ----
            def blk(buf, lo, hi):
                return buf[:].rearrange("p (b w) -> p b w", w=32)[:, :, lo:hi]

            sre, sim_, dre, dim_ = reA, imA, reB, imB
            for si, s in enumerate((1, 2, 4, 8, 16)):
                w = 32 - s
                r0, i0 = blk(sre, s, 32), blk(sim_, s, 32)
                rs, is_ = blk(sre, 0, w), blk(sim_, 0, w)
                rd, id_ = blk(dre, s, 32), blk(dim_, s, 32)
                tt1, tt2 = blk(t1, 0, w), blk(t2, 0, w)
                tt3, tt4 = blk(t3, 0, w), blk(t4, 0, w)
                nc.scalar.copy(blk(dre, 0, s), blk(sre, 0, s))
                nc.vector.tensor_mul(tt1, r0, rs)
                nc.vector.tensor_mul(tt2, i0, is_)
                nc.vector.tensor_sub(rd, tt1, tt2)
                if si < 4:
                    nc.scalar.copy(blk(dim_, 0, s), blk(sim_, 0, s))
                    nc.gpsimd.tensor_mul(tt3, r0, is_)
                    nc.gpsimd.tensor_mul(tt4, i0, rs)
                    nc.gpsimd.tensor_add(id_, tt3, tt4)
                sre, dre = dre, sre
                sim_, dim_ = dim_, sim_
            # final real part is in reB

            # ---- 5. v and g projections (f32r) ----
            pv = ps.tile([128, 512], F32, tag="acc", name="pv")
            for dc in range(DCH):
                nc.tensor.matmul(pv[:], wv_sb[dc], xsr[dc],
                                 start=(dc == 0), stop=(dc == DCH - 1))
            vtile = gte.tile([128, 512], F32, tag="vt", name="vt")
            nc.scalar.copy(vtile[:], pv[:])
            pvn = ps.tile([128, 512], F32, tag="ptr", name="pvn", bufs=1)
            for s in range(4):
                nc.tensor.transpose(
                    pvn[:, s * 128:(s + 1) * 128],
                    vtile[:, s * 128:(s + 1) * 128], ident[:])
            nc.scalar.copy(vb[:, ns], pvn[:])
            pg = ps.tile([128, 512], F32, tag="acc", name="pg")
            for dc in range(DCH):
                nc.tensor.matmul(pg[:], wg_sb[dc], xsr[dc],
                                 start=(dc == 0), stop=(dc == DCH - 1))
            gsC = chk.tile([128, 512], F32, tag="gsC", name="gsC")
            nc.scalar.activation(gsC[:], pg[:],
                                 mybir.ActivationFunctionType.Silu)

            # ---- 4. acr: clip, expand pairs, transpose; 1/acr ----
            acrC = chk.tile([128, 512], F32, tag="acrC", name="acrC")
            krC = chk.tile([128, 512], F32, tag="krC", name="krC")
            for si in range(4):
                ae = aep.tile([128, 128], F32, tag="ae", bufs=2)
                src = reB[:, si * 64:(si + 1) * 64].rearrange(
                    "p (h d) -> p h d", h=2)
                for c in range(2):
                    dst = ae[:].rearrange(
                        "p (h d two) -> p h d two", h=2, two=2)[:, :, :, c]
                    nc.vector.tensor_scalar_max(dst, src, EPS)
                pae = ps.tile([128, 512], F32, tag="ptr", name="pae", bufs=1)
                nc.tensor.transpose(pae[:, 0:128], ae[:], ident[:])
                if si % 2 == 0:
                    nc.vector.tensor_copy(
                        acrC[:, si * 128:(si + 1) * 128], pae[:, 0:128])
                else:
                    nc.scalar.copy(
                        acrC[:, si * 128:(si + 1) * 128], pae[:, 0:128])
            nc.vector.reciprocal_approx_fast(krC[:], acrC[:])

            # ---- 6. q/k projections + decay scaling (bf16 out) ----
            pq = ps.tile([128, 512], F32, tag="acc", name="pq")
            for dc in range(DCH):
                nc.tensor.matmul(pq[:], wq_sb[dc], xsr[dc],
                                 start=(dc == 0), stop=(dc == DCH - 1))
            qt = chk.tile([128, 512], F32R, tag="qt", name="qt")
            nc.vector.tensor_mul(qt[:], pq[:], acrC[:])
            pk = ps.tile([128, 512], F32, tag="acc", name="pk")
            for dc in range(DCH):
                nc.tensor.matmul(pk[:], wk_sb[dc], xsr[dc],
                                 start=(dc == 0), stop=(dc == DCH - 1))
            nc.vector.tensor_mul(kt[:, ns], pk[:], krC[:])

            # ---- 7. causal attention, head-packed on the PE ----
            pouts = [ps.tile([64, 512], F32, tag=f"po{h}",
                                  name=f"po{h}", bufs=1) for h in range(H_LOC)]
            njc = 4 * (c4 + 1)

            def av_mm(ss_pair, jc, lo):
                for h in range(H_LOC):
                    nc.tensor.matmul(
                        pouts[h][:, lo:],
                        vb[:, jc * 128 + h * 64: jc * 128 + (h + 1) * 64],
                        ss_pair[h][:, lo:],
                        start=(jc == 0), stop=(jc == njc - 1))

            pend = []
            for jc in range(njc):
                off = jc - 4 * c4
                lo = max(0, off) * 128
                sss = []
                for h in range(H_LOC):
                    hp = slice(h * 64, (h + 1) * 64)
                    psim = ps.tile([128, 512], F32,
                                   tag=f"psim{h}", name=f"psim{h}",
                                   bufs=2 if h == 0 else 1)
                    nc.tensor.matmul(
                        psim[:, lo:], kt[hp, jc * 128:(jc + 1) * 128],
                        qt[hp, lo:], start=True, stop=True)
                    ss = sse.tile([128, 512], F32R, tag=f"ss{h}",
                                  name=f"ss{h}", bufs=4)
                    eng = nc.vector if (jc + h) % 2 == 0 else nc.scalar
                    if eng is nc.vector:
                        eng.tensor_copy(ss[:, lo:], psim[:, lo:])
                    else:
                        eng.copy(ss[:, lo:], psim[:, lo:])
                    if off >= 0:
                        # zero the future (j > i) triangle of the diagonal
                        # 128x128 block, in place: keep where i_local >= p
                        dg = ss[:, off * 128:(off + 1) * 128]
                        nc.gpsimd.affine_select(
                            out=dg, in_=dg, compare_op=mybir.AluOpType.is_ge,
                            fill=0.0, base=0, pattern=[[1, 128]],
                            channel_multiplier=-1)
                    sss.append(ss)
                pend.append((sss, jc, lo))
                if len(pend) > 2:
                    av_mm(*pend.pop(0))
            for it in pend:
                av_mm(*it)

            # ---- 8. gating + partial out-projection ----
            gt_ = gte.tile([128, 512], F32R, tag="gt")
            for h in range(H_LOC):
                hp = slice(h * 64, (h + 1) * 64)
                nc.vector.tensor_mul(gt_[hp, :], pouts[h][:], gsC[hp, :])
            for ji in range(DCH):
                poj = ps.tile([128, 512], F32, tag=f"psim{ji % 2}",
                              name="poj", bufs=2 if ji % 2 == 0 else 1)
                nc.tensor.matmul(poj[:], wo_sb[ji], gt_[:],
                                 start=True, stop=True)
                ot = ote.tile([128, 512], F32, tag="ot", bufs=3)
                if ji % 2 == 0:
                    nc.scalar.activation(
                        ot[:], poj[:],
                        mybir.ActivationFunctionType.Identity,
                        bias=btile[:, ji:ji + 1])
                else:
                    nc.vector.tensor_scalar_add(
                        ot[:], poj[:], btile[:, ji:ji + 1])
                nc.sync.dma_start(OUT[ji * 128:(ji + 1) * 128, ns], ot[:])

        stage1(0)
        stage1(1)
        stage1(2)
        for c4 in range(NC4):
            if c4 + 3 < NC4:
                stage1(c4 + 3)
            stage2(c4)
    nc.finalize()
    return nc


_NC_CACHE = []


def _get_nc():
    if not _NC_CACHE:
        nc = bacc.Bacc("TRN2", target_bir_lowering=False, debug=False)
        _emit(nc)
        _NC_CACHE.append(nc)
    return _NC_CACHE[0]


def _shard_inputs(x, W_qkv, W_a, W_g, W_out, b_out):
    xT = np.ascontiguousarray(
        np.asarray(x, np.float32).reshape(N, D).T)          # [D, N]
    W_qkv = np.asarray(W_qkv, np.float32)
    W_a = np.asarray(W_a, np.float32)
    W_g = np.asarray(W_g, np.float32)
    W_out = np.asarray(W_out, np.float32)
    b_out = np.asarray(b_out, np.float32)

    # W_a column permutation: within a core's 128 cols, source col
    # h*64 + 2d + c  ->  dest col c*64 + h*32 + d
    perm = np.empty(128, np.int64)
    for c in range(2):
        for h in range(2):
            for d in range(DC):
                perm[c * 64 + h * 32 + d] = h * 64 + 2 * d + c

    in_maps = []
    for r in range(NCORES):
        cs = r * 128
        wq = np.ascontiguousarray(W_qkv[:, cs:cs + 128] * np.float32(DH ** -0.5))
        wk = np.ascontiguousarray(W_qkv[:, D + cs:D + cs + 128])
        wv = np.ascontiguousarray(W_qkv[:, 2 * D + cs:2 * D + cs + 128])
        wa = np.ascontiguousarray(W_a[:, cs:cs + 128][:, perm])
        wg = np.ascontiguousarray(W_g[:, cs:cs + 128])
        wo = np.ascontiguousarray(W_out[cs:cs + 128, :])
        if r == 0:
            bt = np.ascontiguousarray(b_out.reshape(8, 128).T)
        else:
            bt = np.zeros((128, 8), np.float32)
        in_maps.append({
            "XT": xT, "WQ": wq, "WK": wk, "WA": wa, "WV": wv, "WG": wg,
            "WO": wo, "BT": bt,
        })
    return in_maps


def _unshard(results):
    outT = np.zeros((D, N), np.float32)
    for r in results:
        outT += r["OUT"]
    return np.ascontiguousarray(outT.T).reshape(1, N, D)


def run(trace=False, **inputs):
    nc = _get_nc()
    in_maps = _shard_inputs(**inputs)
    res = run_bass_kernel_spmd(nc, in_maps, core_ids=list(range(NCORES)),
                               trace=trace)
    return _unshard(res.results), res


def kernel(**inputs) -> np.ndarray:
    out, _ = run(trace=False, **inputs)
    return out
